# revision 1
# baseline (speedup 1.0000x reference)
"""Trainium2 Bass kernel for nn_BPModel: LSTM encoder -> latent ODE (RK4) -> decoder.

Data-parallel over 8 NeuronCores: batch 4096 -> 512 per core, all parameters
replicated, everything on-chip in [feature, batch] layout.

v2 design notes (from perfetto trace of v1):
- v1 was latency-bound on the LSTM recurrence chain (~6.9us/timestep) with
  ~350ns LDWEIGHTS per matmul: fp32r matmuls must self-load weights, so the
  walrus LDW-dedup pass could not elide anything.
- Weights (stationary operands) are now fp16: half-size loads, and walrus
  elides the second load of back-to-back matmuls sharing a stationary
  (emission keeps same-weight pairs adjacent). Moving operands stay fp32r
  (1 col/cycle at free-size >= 256) for precision.
- LSTM gates PSUM: per stream one [128, 1024] tile (2 banks), regions
  [i|f|o|g] at 256-col offsets: sigmoid(i,f,o) is ONE contiguous 768-col
  Act instr. 2 streams x 2 buffers = all 8 banks -> xproj(t+1) never waits.
- Elementwise split: t1=sig_i*tanh_g on Pool (gpsimd), t2/c'/h on DVE,
  tanh on Act. Whh matmuls skipped at t=0 (h=0).
- ODE: pn/cn trunks merged (stage1 one matmul via stacked [pn1W|cn1W]
  stationary + partition-stacked relu bias; stage2 two K=64 matmuls into one
  psum tile), one [3,256] pn3 matmul, ONE merged Exp with per-partition
  scale/bias APs, K=3 selector matmuls for row broadcasts, true (unpadded)
  weight dims, 2 independent batch streams, relu2 on DVE/Pool.

Engine instructions carry a single HW sync-wait slot; a post-Tile pass
moves excess waits onto same-engine NoOps.
"""

import sys
import numpy as np

for _p in ("/opt/trn_rl_repo",):
    if _p not in sys.path:
        sys.path.insert(0, _p)

import concourse.bass as bass
import concourse.tile as tile
import concourse.mybir as mybir
import concourse.bass_utils as _bu
from concourse.bass_utils import run_bass_kernel_spmd


def _patched_bir_verify_and_optimise(tmpdir, inp="bir.json", outp="file.neff",
                                     arch=None, *, dve_root=None):
    """Same as bass_utils.bir_verify_and_optimise but with walrus LDW
    dedup enabled (redundant LDWEIGHTS elision for back-to-back matmuls
    sharing a stationary operand)."""
    cmd = [
        _bu.get_walrus_driver(),
        "--pass",
        ",".join(["birverifier", "runtime_memory_reservation", "lower_act",
                  "lower_dve", "lower_ap_offset", "codegen", "neff_packager"]),
        "-i", inp,
        "--neff-output-filename", outp,
        "--enable-birsim=true", "--mem-mode=physical", "--policy=0",
        "--enable-ldw-opt=false",
        "--assign-static-dmas-to-sp=false",
        "--dram-page-size=256", "--enable-neff-debug-info=true",
        "--jobs", "8",
        *_bu.get_walrus_args(
            _bu.get_bir_arch(tmpdir, inp) if arch is None else arch,
            tmpdir, dve_root=dve_root),
    ]
    result = _bu.run_command(cmd, cwd=tmpdir)
    if result is not None:
        from pathlib import Path
        (Path(tmpdir) / "log.txt").write_text(result.stdout)
    return f"{tmpdir}/{outp}"


_bu.bir_verify_and_optimise = _patched_bir_verify_and_optimise

F32 = mybir.dt.float32
F32R = mybir.dt.float32r
F16 = mybir.dt.float16
AF = mybir.ActivationFunctionType
ALU = mybir.AluOpType

NCORES = 8
B, T_FULL, D_IN, H, LAT = 4096, 256, 2, 128, 128
BP = B // NCORES          # 512 batch per core
BS = BP // 2              # 256 per stream
N_STEPS = 9
SXT = 16                  # t-slots per xt3 tile (x rows 0..31, ones at 32)

# gate order in PSUM regions: i, f, o, g  (pytorch packs i, f, g, o)
GATE_PERM = (0, 1, 3, 2)
# whh/xproj emission order: g first so tanh(g) unblocks earliest
CI_ORDER = (3, 0, 1, 2)

# fp16 weight tensors (stationary matmul operands)
_W_SPECS = [
    ("Wball", [128, SXT * 512]),
    ("Whh", [128, 512]),
    ("fc1W", [128, 256]),
    ("W1stk", [128, 128]),     # [pn1W | cn1W]
    ("dec1aW", [128, 128]),
    ("fc2W", [128, 256]),
    ("W2stk", [128, 128]),     # rows 0:64 pn2W, 64:128 cn2W
    ("pn3W", [128, 3]),
    ("cn3W", [128, 128]),
    ("selS", [3, 128]),        # rows [1;1;0] -> bcast(rowA+rowB)
    ("selC", [3, 128]),        # rows [0;0;1] -> bcast(rowC)
    ("dec1bW3", [3, 128]),
    ("dec2W", [128, 64]),
    ("dec3W", [64, 2]),
]
# f32 bias / scale tensors
_B_SPECS = [
    ("fc1b2", [128, 2]),
    ("fc2b", [128, 1]),
    ("b1stk", [128, 1]),       # [pn1b(64); cn1b(64)]
    ("pn2b", [128, 1]), ("cn2b", [128, 1]),
    ("cn3b", [128, 1]),
    ("sc3", [3, 1]),           # exp scales [1, -1, -1]
    ("b3", [3, 1]),            # exp biases [b0, -b1, -b2]
    ("b3p", [3, 1]),           # params-exp biases [b0, b1, b2]
    ("dec1b", [128, 1]),
    ("dec2b", [64, 1]),
    ("dec3b", [2, 1]),
]


def _f32(ap):
    return ap.bitcast(F32)


def _legalize_matmul_waits(nc):
    """Engine instructions carry a single HW sync-wait slot (walrus: 'Too
    many sync wait commands'). Move excess waits onto preceding NoOps on the
    same engine queue; engine FIFO order keeps correctness."""
    n_moved = 0
    for fn in nc.m.functions:
        for bb in fn.blocks:
            out = []
            for inst in bb.instructions:
                si = inst.sync_info
                if si is not None and si.on_wait and len(si.on_wait) > 1:
                    waits = list(si.on_wait)
                    for w in waits[:-1]:
                        nop = mybir.InstNoOp(
                            name=nc.get_next_instruction_name(),
                            engine=inst.engine,
                            ins=[], outs=[],
                            sync_info=mybir.SyncInfo(on_wait=[w], on_update=[]),
                        )
                        out.append(nop)
                    si.on_wait = waits[-1:]
                    n_moved += 1
                out.append(inst)
            bb.instructions[:] = out
    return n_moved


def build_program(T=T_FULL, n_steps=N_STEPS, debug=False, legalize=True):
    dt = 1.0 / n_steps
    nxt = (T + SXT - 1) // SXT
    nc = bass.Bass()
    ins = {}
    ins["xt3"] = nc.declare_dram_parameter("xt3", [128, nxt * BP], F16,
                                           isOutput=False)
    for name, shape in _W_SPECS:
        ins[name] = nc.declare_dram_parameter(name, shape, F16, isOutput=False)
    for name, shape in _B_SPECS:
        ins[name] = nc.declare_dram_parameter(name, shape, F32, isOutput=False)
    y_out = nc.declare_dram_parameter("y", [2, BP], F32, isOutput=True)
    if debug:
        dbg_h = nc.declare_dram_parameter("dbg_h", [128, BP], F32, isOutput=True)
        dbg_z0 = nc.declare_dram_parameter("dbg_z0", [128, BP], F32, isOutput=True)
        dbg_zT = nc.declare_dram_parameter("dbg_zT", [128, BP], F32, isOutput=True)
        dbg_pr = nc.declare_dram_parameter("dbg_pr", [3, BP], F32, isOutput=True)

    with tile.TileContext(nc) as tc:
        with (
            tc.tile_pool(name="const", bufs=1) as cp,
            tc.tile_pool(name="state", bufs=2) as st,
        ):
            sb = {}
            sb["xt3"] = cp.tile([128, nxt * BP], F16, tag="xt3", name="xt3")
            nc.sync.dma_start(sb["xt3"][:], ins["xt3"][:])
            for name, shape in _W_SPECS:
                sb[name] = cp.tile(shape, F16, tag=name, name=name)
                nc.sync.dma_start(sb[name][:], ins[name][:])
            for name, shape in _B_SPECS:
                sb[name] = cp.tile(shape, F32, tag=name, name=name)
                nc.sync.dma_start(sb[name][:], ins[name][:])
            par3 = cp.tile([3, BP], F16, tag="par3")

            c = []
            for s in range(2):
                ct = st.tile([128, BS], F32, tag=f"c{s}")
                nc.gpsimd.memset(ct[:], 0.0)
                c.append(ct)

            xt3 = sb["xt3"]
            Wball = sb["Wball"]
            Whh = sb["Whh"]

            # ------------------ LSTM ------------------
            # per-stream gates psum [128, 2048]: one bank per gate [i|f|o|g],
            # 256 cols used of each 512-col bank; 2 streams = 8 banks.
            # g-gate weights are doubled host-side: tanh(g) = 2*sigmoid(2g)-1,
            # so ONE sigmoid instruction covers all four gate regions and the
            # correction runs as cheap DVE ops:
            #   u = 2*sig2g - 1 ; t1 = u*sig_i ; c' = t1 + sig_f*c
            with (
                tc.tile_pool(name="psA", bufs=1, space="PSUM") as gp,
                tc.tile_pool(name="work", bufs=4) as wp,
            ):
                h = [None, None]
                for t in range(T):
                    til, slot = divmod(t, SXT)
                    first = (t == 0)
                    gates = {}
                    for s in range(2):
                        gates[s] = gp.tile([128, 2048], F32, tag=f"g{s}",
                                           name=f"g{s}_{t}")
                    for ci in range(4):
                        for s in range(2):
                            xsl = xt3[:, BP * til + BS * s
                                      : BP * til + BS * (s + 1)]
                            nc.tensor.matmul(
                                gates[s][:, 512 * ci : 512 * ci + BS],
                                Wball[:, 512 * slot + 128 * ci
                                      : 512 * slot + 128 * (ci + 1)],
                                xsl, start=True, stop=first)
                    if not first:
                        for s in range(2):
                            for ci in range(4):
                                nc.tensor.matmul(
                                    gates[s][:, 512 * ci : 512 * ci + BS],
                                    Whh[:, 128 * ci : 128 * (ci + 1)],
                                    h[s][:], start=False, stop=True)
                    sgm = {}
                    for s in range(2):
                        sgm[s] = wp.tile([128, 4 * BS], F16, tag=f"sg{s}",
                                         name=f"sg{s}_{t}")
                        ga = gates[s][:].rearrange("p (r q) -> p r q", r=4)
                        nc.scalar.activation(sgm[s][:], ga[:, 0:4, 0:BS],
                                             AF.Sigmoid)
                    t2 = {}
                    u = {}
                    t1 = {}
                    for s in range(2):
                        # off-path: t2 = sig_f * c on Pool
                        t2[s] = wp.tile([128, BS], F32, tag=f"t2{s}",
                                        name=f"t2{s}_{t}")
                        nc.gpsimd.tensor_tensor(
                            out=t2[s][:], in0=sgm[s][:, BS : 2 * BS],
                            in1=c[s][:], op=ALU.mult)
                    for s in range(2):
                        u[s] = wp.tile([128, BS], F32, tag=f"u{s}",
                                       name=f"u{s}_{t}")
                        nc.vector.tensor_scalar(
                            out=u[s][:], in0=sgm[s][:, 3 * BS : 4 * BS],
                            scalar1=2.0, scalar2=1.0,
                            op0=ALU.mult, op1=ALU.subtract)
                        t1[s] = wp.tile([128, BS], F32, tag=f"t1{s}",
                                        name=f"t1{s}_{t}")
                        nc.vector.tensor_tensor(
                            out=t1[s][:], in0=u[s][:], in1=sgm[s][:, 0:BS],
                            op=ALU.mult)
                    cn = {}
                    for s in range(2):
                        cn[s] = st.tile([128, BS], F32, tag=f"c{s}",
                                        name=f"c{s}_{t}")
                        nc.vector.tensor_tensor(
                            out=cn[s][:], in0=t1[s][:], in1=t2[s][:],
                            op=ALU.add)
                        c[s] = cn[s]
                    tct = {}
                    for s in range(2):
                        tct[s] = wp.tile([128, BS], F16, tag=f"tc{s}",
                                         name=f"tc{s}_{t}")
                        nc.scalar.activation(tct[s][:], cn[s][:], AF.Tanh)
                    for s in range(2):
                        hn_ = st.tile([128, BS], F16, tag=f"h{s}",
                                      name=f"h{s}_{t}")
                        nc.vector.tensor_tensor(
                            out=hn_[:], in0=sgm[s][:, 2 * BS : 3 * BS],
                            in1=tct[s][:], op=ALU.mult)
                        h[s] = hn_

            # ------------- encoder fc + ODE + decoder -------------
            with (
                tc.tile_pool(name="psB", bufs=1, space="PSUM") as pb,
                tc.tile_pool(name="ow", bufs=2) as ow,
            ):
                if debug:
                    for s in range(2):
                        nc.sync.dma_start(
                            dbg_h[:, BS * s : BS * (s + 1)].bitcast(F16)[:, 0:BS],
                            h[s][:])
                # fc1: relu(hN @ fc1W + b); j chunks of the 256-dim output
                r1 = ow.tile([128, 1024], F16, tag="r1")
                for j in range(2):
                    pfc = pb.tile([128, 512], F32, tag=f"pA{j}", name=f"pfc{j}")
                    for s in range(2):
                        nc.tensor.matmul(
                            pfc[:, BS * s : BS * (s + 1)],
                            sb["fc1W"][:, 128 * j : 128 * (j + 1)],
                            h[s][:], start=(s == 0), stop=(s == 1))
                    nc.scalar.activation(
                        r1[:, 512 * j : 512 * (j + 1)], pfc[:], AF.Relu,
                        bias=sb["fc1b2"][:, j : j + 1])
                # fc2 (no relu)
                pz = pb.tile([128, BP], F32, tag="pB0")
                nc.tensor.matmul(pz[:], sb["fc2W"][:, 0:128], r1[:, 0:512],
                                 start=True, stop=False)
                nc.tensor.matmul(pz[:], sb["fc2W"][:, 128:256], r1[:, 512:1024],
                                 start=False, stop=True)
                zs = []
                for s in range(2):
                    zt = ow.tile([128, BS], F16, tag=f"z{s}")
                    nc.vector.tensor_scalar(
                        out=zt[:], in0=pz[:, BS * s : BS * (s + 1)],
                        scalar1=sb["fc2b"][:], scalar2=None, op0=ALU.add)
                    zs.append(zt)
                if debug:
                    for s in range(2):
                        nc.sync.dma_start(dbg_z0[:, BS * s : BS * (s + 1)].bitcast(F16)[:, 0:BS], zs[s][:])

                def odef(zin, s, first=False, ktag="k"):
                    """One odefunc eval for stream s: k = (comp + cn3b
                    - z*(Rp + 1/Rd)) / C, trunks merged. Each matmul output
                    gets its own psum bank (tags pA..pD cycle per stream)."""
                    sl = slice(BS * s, BS * (s + 1))
                    # stage 1: both trunks in one matmul (partition-stacked)
                    p1 = pb.tile([128, 512], F32, tag=f"pA{s}", name=f"p1{s}")
                    nc.tensor.matmul(p1[:, 0:BS], sb["W1stk"][:], zin[:],
                                     start=True, stop=True)
                    s1 = ow.tile([128, BS], F16, tag=f"s1_{s}")
                    nc.scalar.activation(s1[:], p1[:, 0:BS], AF.Relu,
                                         bias=sb["b1stk"][:])
                    # stage 2: two K=64 matmuls, separate banks
                    p2a = pb.tile([128, 512], F32, tag=f"pB{s}", name=f"p2a{s}")
                    nc.tensor.matmul(p2a[:, 0:BS], sb["W2stk"][0:64, :],
                                     s1[0:64, :], start=True, stop=True)
                    p2b = pb.tile([128, 512], F32, tag=f"pC{s}", name=f"p2b{s}")
                    nc.tensor.matmul(p2b[:, 0:BS], sb["W2stk"][64:128, :],
                                     s1[64:128, :], start=True, stop=True)
                    s2p = ow.tile([128, BS], F16, tag=f"s2p{s}")
                    nc.scalar.activation(s2p[:], p2a[:, 0:BS], AF.Relu,
                                         bias=sb["pn2b"][:])
                    s2c = ow.tile([128, BS], F16, tag=f"s2c{s}")
                    nc.scalar.activation(s2c[:], p2b[:, 0:BS], AF.Relu,
                                         bias=sb["cn2b"][:])
                    # stage 3: pn3 -> [3, BS] (bank A); cn3 -> [128, BS] (bank B)
                    p3 = pb.tile([128, 512], F32, tag=f"pA{s}", name=f"p3{s}")
                    pp3 = p3[0:3, 0:BS]
                    nc.tensor.matmul(pp3, sb["pn3W"][:], s2p[:],
                                     start=True, stop=True)
                    pcn = pb.tile([128, 512], F32, tag=f"pB{s}", name=f"pcn{s}")
                    nc.tensor.matmul(pcn[:, 0:BS], sb["cn3W"][:], s2c[:],
                                     start=True, stop=True)
                    # rows = [Rp; 1/Rd; 1/C] = exp(pp3 * [1,-1,-1] + [b0,-b1,-b2])
                    rows = ow.tile([3, BS], F16, tag=f"rw{s}")
                    nc.scalar.activation(rows[:], pp3, AF.Exp,
                                         bias=sb["b3"][:], scale=sb["sc3"][:])
                    if first:
                        nc.scalar.activation(par3[:, sl], pp3, AF.Exp,
                                             bias=sb["b3p"][:], scale=1.0)
                    # Sb = bcast(Rp + 1/Rd) (bank D); Cb = bcast(1/C) (bank C)
                    pbs = pb.tile([128, 512], F32, tag=f"pD{s}", name=f"pbs{s}")
                    nc.tensor.matmul(pbs[:, 0:BS], sb["selS"][:], rows[:],
                                     start=True, stop=True)
                    pbc = pb.tile([128, 512], F32, tag=f"pC{s}", name=f"pbc{s}")
                    nc.tensor.matmul(pbc[:, 0:BS], sb["selC"][:], rows[:],
                                     start=True, stop=True)
                    # k = (comp + cn3b - z*Sb) * Cb
                    d1 = ow.tile([128, BS], F32, tag=f"d1{s}")
                    nc.vector.tensor_tensor(out=d1[:], in0=zin[:],
                                            in1=pbs[:, 0:BS], op=ALU.mult)
                    d2 = ow.tile([128, BS], F32, tag=f"d2{s}")
                    nc.vector.scalar_tensor_tensor(
                        out=d2[:], in0=pcn[:, 0:BS], scalar=sb["cn3b"][:],
                        in1=d1[:], op0=ALU.add, op1=ALU.subtract)
                    k = ow.tile([128, BS], F32, tag=ktag)
                    nc.vector.tensor_tensor(out=k[:], in0=d2[:],
                                            in1=pbc[:, 0:BS], op=ALU.mult)
                    return k

                def sttz(k_in0, scalar, ztile, tag):
                    # f16 out: (k * scalar) + z
                    o = ow.tile([128, BS], F16, tag=tag)
                    nc.vector.scalar_tensor_tensor(
                        out=o[:], in0=k_in0[:], scalar=float(scalar),
                        in1=ztile[:],
                        op0=ALU.mult, op1=ALU.add)
                    return o

                def sttk(in0, scalar, in1, tag):
                    # f32 out: (in0 * scalar) + in1
                    o = ow.tile([128, BS], F32, tag=tag)
                    nc.vector.scalar_tensor_tensor(
                        out=o[:], in0=in0[:], scalar=float(scalar), in1=in1[:],
                        op0=ALU.mult, op1=ALU.add)
                    return o

                def ttp(in0, in1, op, tag):
                    # f32 out on Pool
                    o = ow.tile([128, BS], F32, tag=tag)
                    nc.gpsimd.tensor_tensor(out=o[:], in0=in0[:], in1=in1[:],
                                            op=op)
                    return o

                for step in range(n_steps):
                    for s in range(2):
                        z = zs[s]
                        k1 = odef(z, s, first=(step == 0), ktag=f"k1{s}")
                        za = sttz(k1, dt / 3.0, z, f"za{s}")   # z + dt/3 k1
                        k2 = odef(za, s, ktag=f"k2{s}")
                        u1 = sttk(k1, -1.0 / 3.0, k2, f"u1{s}")  # k2 - k1/3
                        zb = sttz(u1, dt, z, f"za{s}")  # z + dt(k2 - k1/3)
                        k3 = odef(zb, s, ktag=f"k3{s}")
                        u2 = ttp(k1, k2, ALU.subtract, f"u1{s}")
                        u3 = ttp(u2, k3, ALU.add, f"u2{s}")
                        zc2 = sttz(u3, dt, z, f"za{s}")  # z + dt(k1 - k2 + k3)
                        k4 = odef(zc2, s, ktag=f"k4{s}")
                        v1 = ttp(k2, k3, ALU.add, f"u1{s}")
                        v2 = sttk(v1, 3.0, k1, f"u2{s}")  # k1 + 3(k2 + k3)
                        v3 = ttp(v2, k4, ALU.add, f"u1{s}")
                        zs[s] = sttz(v3, dt / 8.0, z, f"z{s}")  # z + dt/8 (..)

                for s in range(2):
                    sl = slice(BS * s, BS * (s + 1))
                    if debug:
                        nc.sync.dma_start(dbg_zT[:, sl].bitcast(F16)[:, 0:BS], zs[s][:])
                        if s == 0:
                            nc.sync.dma_start(dbg_pr[:].bitcast(F16)[:, 0:BP], par3[:])
                    # decoder: zc = [zT ; params]
                    pd1 = pb.tile([128, 512], F32, tag=f"pA{s}",
                                  name=f"pd1{s}")
                    nc.tensor.matmul(pd1[:, 0:BS], sb["dec1aW"][:], zs[s][:],
                                     start=True, stop=False)
                    nc.tensor.matmul(pd1[:, 0:BS], sb["dec1bW3"][:],
                                     par3[:, sl], start=False, stop=True)
                    sd1 = ow.tile([128, BS], F16, tag=f"sd1{s}")
                    nc.scalar.activation(sd1[:], pd1[:, 0:BS], AF.Relu,
                                         bias=sb["dec1b"][:])
                    pd2 = pb.tile([128, 512], F32, tag=f"pB{s}",
                                  name=f"pd2{s}")
                    nc.tensor.matmul(pd2[0:64, 0:BS], sb["dec2W"][:], sd1[:],
                                     start=True, stop=True)
                    sd2 = ow.tile([64, BS], F16, tag=f"sd2{s}")
                    nc.scalar.activation(sd2[:], pd2[0:64, 0:BS], AF.Relu,
                                         bias=sb["dec2b"][:])
                    pd3 = pb.tile([128, 512], F32, tag=f"pC{s}",
                                  name=f"pd3{s}")
                    nc.tensor.matmul(pd3[0:2, 0:BS], sb["dec3W"][:], sd2[:],
                                     start=True, stop=True)
                    yt = ow.tile([2, BS], F32, tag=f"y{s}")
                    nc.vector.tensor_scalar(out=yt[:], in0=pd3[0:2, 0:BS],
                                            scalar1=sb["dec3b"][:],
                                            scalar2=None, op0=ALU.add)
                    nc.sync.dma_start(y_out[:, sl], yt[:])

    if legalize:
        _legalize_matmul_waits(nc)
    return nc


def prep_inputs(inputs, T=T_FULL):
    """Host-side marshaling: shard x, build xt3/Wball layouts, repack weights."""
    nxt = (T + SXT - 1) // SXT
    f = lambda a: np.ascontiguousarray(a, dtype=np.float32)
    f16 = lambda a: np.ascontiguousarray(a, dtype=np.float16)
    x = f(inputs["x"])                      # [B, T, 2]
    Wih = f(inputs["lstm_Wih"])             # [2, 512]
    Whh = f(inputs["lstm_Whh"])             # [128, 512]
    bsum = f(inputs["lstm_bih"] + inputs["lstm_bhh"])   # [512]

    # permute gate chunks (i, f, g, o) -> (i, f, o, g)
    def permc(w):
        chunks = [w[..., 128 * cc : 128 * (cc + 1)] for cc in GATE_PERM]
        return np.concatenate(chunks, axis=-1)

    Wih_p, Whh_p, bsum_p = permc(Wih), permc(Whh), permc(bsum)
    # g-gate doubled: tanh(g) computed as 2*sigmoid(2g)-1 on-chip
    Wih_p[:, 384:512] *= 2.0
    Whh_p[:, 384:512] *= 2.0
    bsum_p[384:512] *= 2.0

    # Wball: [128, SXT*512]; slot s: rows 2s,2s+1 = Wih rows, row 32 = bias
    Wball = np.zeros((128, SXT * 512), dtype=np.float32)
    for s in range(SXT):
        Wball[2 * s, 512 * s : 512 * (s + 1)] = Wih_p[0]
        Wball[2 * s + 1, 512 * s : 512 * (s + 1)] = Wih_p[1]
        Wball[32, 512 * s : 512 * (s + 1)] = bsum_p

    # xt3 per core: [128, nxt*BP]; tile t//SXT, x rows 2(t%SXT), ones row 32
    xt3_all = np.zeros((NCORES, 128, nxt * BP), dtype=np.float16)
    xs = x.reshape(NCORES, BP, T, 2)
    for core in range(NCORES):
        xc = xs[core]                       # [BP, T, 2]
        for t in range(T):
            til, slot = divmod(t, SXT)
            col0 = BP * til
            xt3_all[core, 2 * slot, col0 : col0 + BP] = xc[:, t, 0]
            xt3_all[core, 2 * slot + 1, col0 : col0 + BP] = xc[:, t, 1]
        xt3_all[core, 32, :] = 1.0

    fc1_b = f(inputs["fc1_b"])
    fc2_W = f(inputs["fc2_W"])
    pn3_b = f(inputs["pn3_b"])
    dec1_W = f(inputs["dec1_W"])            # [131, 128]

    selS = np.zeros((3, 128), dtype=np.float32)
    selS[0, :] = 1.0
    selS[1, :] = 1.0
    selC = np.zeros((3, 128), dtype=np.float32)
    selC[2, :] = 1.0

    common = {
        "Wball": f16(Wball),
        "Whh": f16(Whh_p),
        "fc1W": f16(inputs["fc1_W"]),
        "fc1b2": f(fc1_b.reshape(2, 128).T),
        "fc2W": f16(np.concatenate([fc2_W[0:128], fc2_W[128:256]], axis=1)),
        "fc2b": f(inputs["fc2_b"][:, None]),
        "W1stk": f16(np.concatenate(
            [inputs["pn1_W"], inputs["cn1_W"]], axis=1)),   # [128, 64+64]
        "b1stk": f(np.concatenate(
            [inputs["pn1_b"], inputs["cn1_b"]])[:, None]),
        "W2stk": f16(np.concatenate(
            [inputs["pn2_W"], inputs["cn2_W"]], axis=0)),  # [128, 128]
        "pn2b": f(inputs["pn2_b"][:, None]),
        "cn2b": f(inputs["cn2_b"][:, None]),
        "pn3W": f16(inputs["pn3_W"]),        # [128, 3]
        "cn3W": f16(inputs["cn3_W"]),        # [128, 128]
        "cn3b": f(inputs["cn3_b"][:, None]),
        "sc3": np.array([[1.0], [-1.0], [-1.0]], dtype=np.float32),
        "b3": np.array([[pn3_b[0]], [-pn3_b[1]], [-pn3_b[2]]],
                       dtype=np.float32),
        "b3p": f(pn3_b[:, None]),
        "selS": f16(selS),
        "selC": f16(selC),
        "dec1aW": f16(dec1_W[0:128]),
        "dec1bW3": f16(dec1_W[128:131]),
        "dec1b": f(inputs["dec1_b"][:, None]),
        "dec2W": f16(inputs["dec2_W"]),      # [128, 64]
        "dec2b": f(inputs["dec2_b"][:, None]),
        "dec3W": f16(inputs["dec3_W"]),      # [64, 2]
        "dec3b": f(inputs["dec3_b"][:, None]),
    }

    in_maps = []
    for core in range(NCORES):
        m = dict(common)
        m["xt3"] = xt3_all[core]
        in_maps.append(m)
    return in_maps


_PROGRAM = None


def get_program():
    global _PROGRAM
    if _PROGRAM is None:
        _PROGRAM = build_program()
    return _PROGRAM


def run(inputs, **kwargs):
    nc = get_program()
    in_maps = prep_inputs(inputs)
    res = run_bass_kernel_spmd(nc, in_maps, list(range(NCORES)), **kwargs)
    outs = [res.results[i]["y"] for i in range(NCORES)]   # each [2, BP]
    y = np.concatenate([o.T for o in outs], axis=0).astype(np.float32)  # [B, 2]
    return y, res


def kernel(**inputs):
    y, _ = run(inputs)
    return y



# revision 5
# speedup vs baseline: 1.0972x; 1.0972x over previous
"""Trainium2 Bass kernel for nn_BPModel: LSTM encoder -> latent ODE (RK4) -> decoder.

Data-parallel over 8 NeuronCores: batch 4096 -> 512 per core, all parameters
replicated, everything on-chip in [feature, batch] layout.

v3 design notes (from perfetto trace of v2 @ 1586us):
- v2's step period (5.16us) equaled the per-stream recurrence LOOP latency:
  sig(1113) -> slow DVE chain (u=776ns!) -> tanh -> h -> cold-PE Whh (852).
  PE never left HAM K=4/8 (half clock); Act idle 37%.
- xproj now uses 4 CONCURRENT K=32 row-strip matmuls (tile_position) with x
  replicated across 4 partition strips (15 t-slots/tile + ones row at 32k+30):
  cuts cold-PE xproj cost ~4x so the tensor engine is off the critical path.
- Gates PSUM: per stream [128, 1024] (2 banks), regions [i|f|o|2g] at 256-col
  offsets, pool bufs=2 (4 tiles = all 8 banks) -> xproj(t+1) fully overlaps.
  ONE contiguous 1024-col sigmoid per stream (g doubled: tanh(g)=2*sig(2g)-1).
- Whole DVE chain in f16 (2x_1P tensor_tensor, 4x tensor_scalar); cell state
  c kept in f16 (forget-gate contraction bounds the rounding error).
- ODE phase: stage-interleaved emission across the two batch streams (v2
  emitted stream1's whole RK4 chain after stream0's -> serial), cn-trunk relu
  moved to DVE (add+max), z-combines fused to ONE on-chain op per k via
  off-chain STT precomputes.

Engine instructions carry a single HW sync-wait slot; a post-Tile pass
moves excess waits onto same-engine NoOps.
"""

import sys
import numpy as np

for _p in ("/opt/trn_rl_repo",):
    if _p not in sys.path:
        sys.path.insert(0, _p)

import concourse.bass as bass
import concourse.tile as tile
import concourse.mybir as mybir
import concourse.bass_utils as _bu
from concourse.bass_utils import run_bass_kernel_spmd


def _patched_bir_verify_and_optimise(tmpdir, inp="bir.json", outp="file.neff",
                                     arch=None, *, dve_root=None):
    """Same as bass_utils.bir_verify_and_optimise but with explicit pass list
    (keeps LDW handling deterministic)."""
    cmd = [
        _bu.get_walrus_driver(),
        "--pass",
        ",".join(["birverifier", "runtime_memory_reservation", "lower_act",
                  "lower_dve", "lower_ap_offset", "codegen", "neff_packager"]),
        "-i", inp,
        "--neff-output-filename", outp,
        "--enable-birsim=true", "--mem-mode=physical", "--policy=0",
        "--enable-ldw-opt=false",
        "--assign-static-dmas-to-sp=false",
        "--dram-page-size=256", "--enable-neff-debug-info=true",
        "--jobs", "8",
        *_bu.get_walrus_args(
            _bu.get_bir_arch(tmpdir, inp) if arch is None else arch,
            tmpdir, dve_root=dve_root),
    ]
    result = _bu.run_command(cmd, cwd=tmpdir)
    if result is not None:
        from pathlib import Path
        (Path(tmpdir) / "log.txt").write_text(result.stdout)
    return f"{tmpdir}/{outp}"


_bu.bir_verify_and_optimise = _patched_bir_verify_and_optimise

F32 = mybir.dt.float32
F32R = mybir.dt.float32r
F16 = mybir.dt.float16
AF = mybir.ActivationFunctionType
ALU = mybir.AluOpType

NCORES = 8
B, T_FULL, D_IN, H, LAT = 4096, 256, 2, 128, 128
BP = B // NCORES          # 512 batch per core
BS = BP // 2              # 256 per stream
N_STEPS = 9
SXT = 15                  # t-slots per xt3 strip (rows 2j,2j+1; ones at +30)

# gate order in PSUM regions: i, f, o, g  (pytorch packs i, f, g, o)
GATE_PERM = (0, 1, 3, 2)

# fp16 weight tensors (stationary matmul operands)
_W_SPECS = [
    ("Wball", [128, SXT * 128]),   # strip ci rows 32ci+2j/+1, bias 32ci+30
    ("Whh", [128, 512]),
    ("fc1W", [128, 256]),
    ("W1stk", [128, 128]),     # [pn1W | cn1W]
    ("dec1aW", [128, 128]),
    ("fc2W", [128, 256]),
    ("W2stk", [128, 128]),     # rows 0:64 pn2W, 64:128 cn2W
    ("pn3W", [128, 3]),
    ("cn3W", [128, 128]),
    ("selS", [3, 128]),        # rows [1;1;0] -> bcast(rowA+rowB)
    ("selC", [3, 128]),        # rows [0;0;1] -> bcast(rowC)
    ("dec1bW3", [3, 128]),
    ("dec2W", [128, 64]),
    ("dec3W", [64, 2]),
]
# f32 bias / scale tensors
_B_SPECS = [
    ("fc1b2", [128, 2]),
    ("fc2b", [128, 1]),
    ("b1stk", [128, 1]),       # [pn1b(64); cn1b(64)]
    ("pn2b", [128, 1]), ("cn2b", [128, 1]),
    ("cn3b", [128, 1]),
    ("sc3", [3, 1]),           # exp scales [1, -1, -1]
    ("b3", [3, 1]),            # exp biases [b0, -b1, -b2]
    ("b3p", [3, 1]),           # params-exp biases [b0, b1, b2]
    ("dec1b", [128, 1]),
    ("dec2b", [64, 1]),
    ("dec3b", [2, 1]),
]


def _legalize_matmul_waits(nc):
    """Engine instructions carry a single HW sync-wait slot (walrus: 'Too
    many sync wait commands'). Move excess waits onto preceding NoOps on the
    same engine queue; engine FIFO order keeps correctness."""
    n_moved = 0
    for fn in nc.m.functions:
        for bb in fn.blocks:
            out = []
            for inst in bb.instructions:
                si = inst.sync_info
                if si is not None and si.on_wait and len(si.on_wait) > 1:
                    waits = list(si.on_wait)
                    for w in waits[:-1]:
                        nop = mybir.InstNoOp(
                            name=nc.get_next_instruction_name(),
                            engine=inst.engine,
                            ins=[], outs=[],
                            sync_info=mybir.SyncInfo(on_wait=[w], on_update=[]),
                        )
                        out.append(nop)
                    si.on_wait = waits[-1:]
                    n_moved += 1
                out.append(inst)
            bb.instructions[:] = out
    return n_moved


def build_program(T=T_FULL, n_steps=N_STEPS, debug=False, legalize=True):
    dt = 1.0 / n_steps
    nxt = (T + SXT - 1) // SXT
    nc = bass.Bass()
    ins = {}
    ins["xt3"] = nc.declare_dram_parameter("xt3", [128, nxt * BP], F16,
                                           isOutput=False)
    for name, shape in _W_SPECS:
        ins[name] = nc.declare_dram_parameter(name, shape, F16, isOutput=False)
    for name, shape in _B_SPECS:
        ins[name] = nc.declare_dram_parameter(name, shape, F32, isOutput=False)
    y_out = nc.declare_dram_parameter("y", [2, BP], F32, isOutput=True)
    if debug:
        dbg_h = nc.declare_dram_parameter("dbg_h", [128, BP], F32, isOutput=True)
        dbg_z0 = nc.declare_dram_parameter("dbg_z0", [128, BP], F32, isOutput=True)
        dbg_zT = nc.declare_dram_parameter("dbg_zT", [128, BP], F32, isOutput=True)
        dbg_pr = nc.declare_dram_parameter("dbg_pr", [3, BP], F32, isOutput=True)

    with tile.TileContext(nc) as tc:
        with (
            tc.tile_pool(name="const", bufs=1) as cp,
            tc.tile_pool(name="state", bufs=2) as st,
        ):
            sb = {}
            sb["xt3"] = cp.tile([128, nxt * BP], F16, tag="xt3", name="xt3")
            nc.sync.dma_start(sb["xt3"][:], ins["xt3"][:])
            for name, shape in _W_SPECS:
                sb[name] = cp.tile(shape, F16, tag=name, name=name)
                nc.sync.dma_start(sb[name][:], ins[name][:])
            for name, shape in _B_SPECS:
                sb[name] = cp.tile(shape, F32, tag=name, name=name)
                nc.sync.dma_start(sb[name][:], ins[name][:])
            par3 = cp.tile([3, BP], F16, tag="par3")

            xt3 = sb["xt3"]
            Wball = sb["Wball"]
            Whh = sb["Whh"]

            # ------------------ LSTM ------------------
            # Per stream gates PSUM [128, 2048] (4 banks), regions
            # [i|f|o|2g] at 512-col offsets (256 used per bank) so each
            # xproj row-strip owns its own PSUM bank (concurrent row tiles
            # MUST NOT write the same bank simultaneously -> HW hang).
            # xproj: 4 concurrent K=32 row-strip matmuls (strip ci holds the
            # ci-th gate chunk; x + ones replicated per strip).
            # tanh(g) = 2*sigmoid(2g)-1 (g weights doubled host-side) so ONE
            # 1024-col (strided) sigmoid covers all four gate regions.
            with (
                tc.tile_pool(name="psA", bufs=1, space="PSUM") as gp,
                tc.tile_pool(name="work", bufs=4) as wp,
            ):
                def emit_xproj(t, gates):
                    til, j = divmod(t, SXT)
                    first = (t == 0)
                    for ci in range(4):
                        stat = Wball[32 * ci: 32 * ci + 32,
                                     128 * j: 128 * (j + 1)]
                        for s in range(2):
                            mov = xt3[32 * ci: 32 * ci + 32,
                                      BP * til + BS * s: BP * til + BS * (s + 1)]
                            nc.tensor.matmul(
                                gates[s][:, 512 * ci: 512 * ci + BS],
                                stat, mov, start=True, stop=first,
                                tile_position=(32 * ci, 0))

                h = [None, None]
                c = [None, None]
                gates_cur = None
                for t in range(T):
                    if t == 0:
                        gates_cur = {}
                        for s in range(2):
                            gates_cur[s] = gp.tile([128, 2048], F32,
                                                   tag=f"g{s}", name=f"g{s}_0")
                        emit_xproj(0, gates_cur)
                    # Whh for this step (ready when h[s] lands)
                    if t > 0:
                        for s in range(2):
                            for ci in range(4):
                                nc.tensor.matmul(
                                    gates_cur[s][:, 512 * ci: 512 * ci + BS],
                                    Whh[:, 128 * ci: 128 * (ci + 1)],
                                    h[s][:], start=False, stop=True)
                    # prefetch next step's x-projection (independent work;
                    # bufs=1 -> waits only for sig(t)'s read of the banks)
                    gates_nxt = None
                    if t + 1 < T:
                        gates_nxt = {}
                        for s in range(2):
                            gates_nxt[s] = gp.tile([128, 2048], F32,
                                                   tag=f"g{s}",
                                                   name=f"g{s}_{t + 1}")
                        emit_xproj(t + 1, gates_nxt)
                    sgm = {}
                    for s in range(2):
                        sgm[s] = wp.tile([128, 1024], F16, tag=f"sg{s}",
                                         name=f"sg{s}_{t}")
                        ga = gates_cur[s][:].rearrange("p (r q) -> p r q", r=4)
                        nc.scalar.activation(sgm[s][:], ga[:, 0:4, 0:BS],
                                             AF.Sigmoid)
                    for s in range(2):
                        if t > 0:
                            # off-path: t2 = sig_f * c on Pool
                            t2 = wp.tile([128, BS], F16, tag=f"t2{s}",
                                         name=f"t2{s}_{t}")
                            nc.gpsimd.tensor_tensor(
                                out=t2[:], in0=sgm[s][:, 256:512],
                                in1=c[s][:], op=ALU.mult)
                        u = wp.tile([128, BS], F16, tag=f"u{s}",
                                    name=f"u{s}_{t}")
                        nc.vector.tensor_scalar(
                            out=u[:], in0=sgm[s][:, 768:1024],
                            scalar1=2.0, scalar2=1.0,
                            op0=ALU.mult, op1=ALU.subtract)
                        cn = st.tile([128, BS], F16, tag=f"c{s}",
                                     name=f"c{s}_{t}")
                        if t > 0:
                            t1 = wp.tile([128, BS], F16, tag=f"t1{s}",
                                         name=f"t1{s}_{t}")
                            nc.vector.tensor_tensor(
                                out=t1[:], in0=u[:], in1=sgm[s][:, 0:256],
                                op=ALU.mult)
                            nc.vector.tensor_tensor(
                                out=cn[:], in0=t1[:], in1=t2[:], op=ALU.add)
                        else:
                            nc.vector.tensor_tensor(
                                out=cn[:], in0=u[:], in1=sgm[s][:, 0:256],
                                op=ALU.mult)
                        c[s] = cn
                    for s in range(2):
                        tct = wp.tile([128, BS], F16, tag=f"tc{s}",
                                      name=f"tc{s}_{t}")
                        nc.scalar.activation(tct[:], c[s][:], AF.Tanh)
                        hn = st.tile([128, BS], F16, tag=f"h{s}",
                                     name=f"h{s}_{t}")
                        nc.vector.tensor_tensor(
                            out=hn[:], in0=sgm[s][:, 512:768], in1=tct[:],
                            op=ALU.mult)
                        h[s] = hn
                    gates_cur = gates_nxt

            # ------------- encoder fc + ODE + decoder -------------
            with (
                tc.tile_pool(name="psB", bufs=1, space="PSUM") as pb,
                tc.tile_pool(name="ow", bufs=2) as ow,
            ):
                if debug:
                    for s in range(2):
                        nc.sync.dma_start(
                            dbg_h[:, BS * s: BS * (s + 1)].bitcast(F16)[:, 0:BS],
                            h[s][:])
                # fc1: relu(hN @ fc1W + b); j chunks of the 256-dim output
                r1 = ow.tile([128, 1024], F16, tag="r1")
                for j in range(2):
                    pfc = pb.tile([128, 512], F32, tag=f"pA{j}", name=f"pfc{j}")
                    for s in range(2):
                        nc.tensor.matmul(
                            pfc[:, BS * s: BS * (s + 1)],
                            sb["fc1W"][:, 128 * j: 128 * (j + 1)],
                            h[s][:], start=(s == 0), stop=(s == 1))
                    nc.scalar.activation(
                        r1[:, 512 * j: 512 * (j + 1)], pfc[:], AF.Relu,
                        bias=sb["fc1b2"][:, j: j + 1])
                # fc2 (no relu)
                pz = pb.tile([128, BP], F32, tag="pB0")
                nc.tensor.matmul(pz[:], sb["fc2W"][:, 0:128], r1[:, 0:512],
                                 start=True, stop=False)
                nc.tensor.matmul(pz[:], sb["fc2W"][:, 128:256], r1[:, 512:1024],
                                 start=False, stop=True)
                zs = []
                for s in range(2):
                    zt = ow.tile([128, BS], F16, tag=f"z{s}")
                    nc.vector.tensor_scalar(
                        out=zt[:], in0=pz[:, BS * s: BS * (s + 1)],
                        scalar1=sb["fc2b"][:], scalar2=None, op0=ALU.add)
                    zs.append(zt)
                if debug:
                    for s in range(2):
                        nc.sync.dma_start(
                            dbg_z0[:, BS * s: BS * (s + 1)].bitcast(F16)[:, 0:BS],
                            zs[s][:])

                def odef_pair(zin, first=False, ktag="k"):
                    """One odefunc eval for BOTH streams, stage-interleaved:
                    k = (comp + cn3b - z*(Rp + 1/Rd)) / C.  pn/cn trunks
                    merged; psum tags pA..pD cycle per stream."""
                    p1, s1v = {}, {}
                    for s in range(2):
                        p1[s] = pb.tile([128, 512], F32, tag=f"pA{s}",
                                        name=f"p1{s}")
                        nc.tensor.matmul(p1[s][:, 0:BS], sb["W1stk"][:],
                                         zin[s][:], start=True, stop=True)
                    for s in range(2):
                        s1v[s] = ow.tile([128, BS], F16, tag=f"s1_{s}", name=f"s1_{s}")
                        nc.scalar.activation(s1v[s][:], p1[s][:, 0:BS],
                                             AF.Relu, bias=sb["b1stk"][:])
                    p2a, p2b = {}, {}
                    for s in range(2):
                        # two K=64 matmuls: distinct row strips -> concurrent
                        p2a[s] = pb.tile([128, 512], F32, tag=f"pB{s}",
                                         name=f"p2a{s}")
                        nc.tensor.matmul(p2a[s][:, 0:BS], sb["W2stk"][0:64, :],
                                         s1v[s][0:64, :], start=True, stop=True)
                        p2b[s] = pb.tile([128, 512], F32, tag=f"pC{s}",
                                         name=f"p2b{s}")
                        nc.tensor.matmul(p2b[s][:, 0:BS],
                                         sb["W2stk"][64:128, :],
                                         s1v[s][64:128, :], start=True,
                                         stop=True)
                    s2p, s2c = {}, {}
                    for s in range(2):
                        s2p[s] = ow.tile([128, BS], F16, tag=f"s2p{s}", name=f"s2p{s}")
                        nc.scalar.activation(s2p[s][:], p2a[s][:, 0:BS],
                                             AF.Relu, bias=sb["pn2b"][:])
                        # cn-trunk relu on DVE: max(x + b, 0)
                        s2c[s] = ow.tile([128, BS], F16, tag=f"s2c{s}", name=f"s2c{s}")
                        nc.vector.tensor_scalar(
                            out=s2c[s][:], in0=p2b[s][:, 0:BS],
                            scalar1=sb["cn2b"][:], scalar2=0.0,
                            op0=ALU.add, op1=ALU.max)
                    p3, pcn = {}, {}
                    for s in range(2):
                        p3[s] = pb.tile([128, 512], F32, tag=f"pA{s}",
                                        name=f"p3{s}")
                        nc.tensor.matmul(p3[s][0:3, 0:BS], sb["pn3W"][:],
                                         s2p[s][:], start=True, stop=True)
                        pcn[s] = pb.tile([128, 512], F32, tag=f"pB{s}",
                                         name=f"pcn{s}")
                        nc.tensor.matmul(pcn[s][:, 0:BS], sb["cn3W"][:],
                                         s2c[s][:], start=True, stop=True)
                    rows = {}
                    for s in range(2):
                        # rows = [Rp; 1/Rd; 1/C] = exp(pp3*[1,-1,-1] + b3)
                        rows[s] = ow.tile([3, BS], F16, tag=f"rw{s}", name=f"rw{s}")
                        nc.scalar.activation(rows[s][:], p3[s][0:3, 0:BS],
                                             AF.Exp, bias=sb["b3"][:],
                                             scale=sb["sc3"][:])
                        if first:
                            nc.scalar.activation(
                                par3[:, BS * s: BS * (s + 1)],
                                p3[s][0:3, 0:BS], AF.Exp,
                                bias=sb["b3p"][:], scale=1.0)
                    pbs, pbc = {}, {}
                    for s in range(2):
                        pbs[s] = pb.tile([128, 512], F32, tag=f"pD{s}",
                                         name=f"pbs{s}")
                        nc.tensor.matmul(pbs[s][:, 0:BS], sb["selS"][:],
                                         rows[s][:], start=True, stop=True)
                        pbc[s] = pb.tile([128, 512], F32, tag=f"pC{s}",
                                         name=f"pbc{s}")
                        nc.tensor.matmul(pbc[s][:, 0:BS], sb["selC"][:],
                                         rows[s][:], start=True, stop=True)
                    k = {}
                    for s in range(2):
                        # k = (comp + cn3b - z*Sb) * Cb
                        d1 = ow.tile([128, BS], F32, tag=f"d1{s}", name=f"d1{s}")
                        nc.vector.tensor_tensor(out=d1[:], in0=zin[s][:],
                                                in1=pbs[s][:, 0:BS],
                                                op=ALU.mult)
                        d2 = ow.tile([128, BS], F32, tag=f"d2{s}", name=f"d2{s}")
                        nc.vector.scalar_tensor_tensor(
                            out=d2[:], in0=pcn[s][:, 0:BS],
                            scalar=sb["cn3b"][:], in1=d1[:],
                            op0=ALU.add, op1=ALU.subtract)
                        k[s] = ow.tile([128, BS], F32, tag=f"{ktag}{s}", name=f"{ktag}{s}")
                        nc.vector.tensor_tensor(out=k[s][:], in0=d2[:],
                                                in1=pbc[s][:, 0:BS],
                                                op=ALU.mult)
                    return k

                def stt(in0, scalar, in1, tag, dtype=F32, eng="v"):
                    # out = in0 * scalar + in1
                    o = ow.tile([128, BS], dtype, tag=tag, name=tag)
                    nc.vector.scalar_tensor_tensor(
                        out=o[:], in0=in0[:], scalar=float(scalar), in1=in1[:],
                        op0=ALU.mult, op1=ALU.add)
                    return o

                def ttp(in0, in1, op, tag):
                    # f32 out on Pool (off-chain combines)
                    o = ow.tile([128, BS], F32, tag=tag, name=tag)
                    nc.gpsimd.tensor_tensor(out=o[:], in0=in0[:], in1=in1[:],
                                            op=op)
                    return o

                for step in range(n_steps):
                    z = list(zs)
                    k1 = odef_pair(z, first=(step == 0), ktag="k1")
                    # za = z + dt/3 * k1   (on-chain, one STT per stream)
                    za = [stt(k1[s], dt / 3.0, z[s], f"za{s}", F16)
                          for s in range(2)]
                    # off-chain while k2 evals: w1 = z - dt/3*k1
                    w1 = [stt(k1[s], -dt / 3.0, z[s], f"w1{s}")
                          for s in range(2)]
                    k2 = odef_pair(za, ktag="k2")
                    # zb = w1 + dt*k2 = z + dt*(k2 - k1/3)
                    zb = [stt(k2[s], dt, w1[s], f"za{s}", F16)
                          for s in range(2)]
                    # off-chain while k3 evals: u2 = k1 - k2 (Pool),
                    # w2 = z + dt*u2
                    u2 = [ttp(k1[s], k2[s], ALU.subtract, f"u2{s}")
                          for s in range(2)]
                    w2 = [stt(u2[s], dt, z[s], f"w1{s}") for s in range(2)]
                    k3 = odef_pair(zb, ktag="k3")
                    # zc = w2 + dt*k3 = z + dt*(k1 - k2 + k3)
                    zc = [stt(k3[s], dt, w2[s], f"za{s}", F16)
                          for s in range(2)]
                    # off-chain while k4 evals: v1 = k2 + k3 (Pool),
                    # v2 = k1 + 3*v1, w3 = z + dt/8*v2
                    v1 = [ttp(k2[s], k3[s], ALU.add, f"u2{s}")
                          for s in range(2)]
                    v2 = [stt(v1[s], 3.0, k1[s], f"w1{s}") for s in range(2)]
                    w3 = [stt(v2[s], dt / 8.0, z[s], f"w3{s}")
                          for s in range(2)]
                    k4 = odef_pair(zc, ktag="k4")
                    # z' = w3 + dt/8*k4
                    zs = [stt(k4[s], dt / 8.0, w3[s], f"z{s}", F16)
                          for s in range(2)]

                for s in range(2):
                    sl = slice(BS * s, BS * (s + 1))
                    if debug:
                        nc.sync.dma_start(
                            dbg_zT[:, sl].bitcast(F16)[:, 0:BS], zs[s][:])
                        if s == 0:
                            nc.sync.dma_start(
                                dbg_pr[:].bitcast(F16)[:, 0:BP], par3[:])
                    # decoder: zc = [zT ; params]
                    pd1 = pb.tile([128, 512], F32, tag=f"pA{s}",
                                  name=f"pd1{s}")
                    nc.tensor.matmul(pd1[:, 0:BS], sb["dec1aW"][:], zs[s][:],
                                     start=True, stop=False)
                    nc.tensor.matmul(pd1[:, 0:BS], sb["dec1bW3"][:],
                                     par3[:, sl], start=False, stop=True)
                    sd1 = ow.tile([128, BS], F16, tag=f"sd1{s}")
                    nc.scalar.activation(sd1[:], pd1[:, 0:BS], AF.Relu,
                                         bias=sb["dec1b"][:])
                    pd2 = pb.tile([128, 512], F32, tag=f"pB{s}",
                                  name=f"pd2{s}")
                    nc.tensor.matmul(pd2[0:64, 0:BS], sb["dec2W"][:], sd1[:],
                                     start=True, stop=True)
                    sd2 = ow.tile([64, BS], F16, tag=f"sd2{s}")
                    nc.scalar.activation(sd2[:], pd2[0:64, 0:BS], AF.Relu,
                                         bias=sb["dec2b"][:])
                    pd3 = pb.tile([128, 512], F32, tag=f"pC{s}",
                                  name=f"pd3{s}")
                    nc.tensor.matmul(pd3[0:2, 0:BS], sb["dec3W"][:], sd2[:],
                                     start=True, stop=True)
                    yt = ow.tile([2, BS], F32, tag=f"y{s}")
                    nc.vector.tensor_scalar(out=yt[:], in0=pd3[0:2, 0:BS],
                                            scalar1=sb["dec3b"][:],
                                            scalar2=None, op0=ALU.add)
                    nc.sync.dma_start(y_out[:, sl], yt[:])

    if legalize:
        _legalize_matmul_waits(nc)
    return nc


def prep_inputs(inputs, T=T_FULL):
    """Host-side marshaling: shard x, build strip-replicated xt3/Wball
    layouts, repack weights."""
    nxt = (T + SXT - 1) // SXT
    f = lambda a: np.ascontiguousarray(a, dtype=np.float32)
    f16 = lambda a: np.ascontiguousarray(a, dtype=np.float16)
    x = f(inputs["x"])                      # [B, T, 2]
    Wih = f(inputs["lstm_Wih"])             # [2, 512]
    Whh = f(inputs["lstm_Whh"])             # [128, 512]
    bsum = f(inputs["lstm_bih"] + inputs["lstm_bhh"])   # [512]

    # permute gate chunks (i, f, g, o) -> (i, f, o, g)
    def permc(w):
        chunks = [w[..., 128 * cc: 128 * (cc + 1)] for cc in GATE_PERM]
        return np.concatenate(chunks, axis=-1)

    Wih_p, Whh_p, bsum_p = permc(Wih), permc(Whh), permc(bsum)
    # g-gate doubled: tanh(g) = 2*sigmoid(2g)-1 on-chip
    Wih_p[:, 384:512] *= 2.0
    Whh_p[:, 384:512] *= 2.0
    bsum_p[384:512] *= 2.0

    # Wball: [128, SXT*128]; strip ci / slot j: rows 32ci+2j, 32ci+2j+1 hold
    # Wih rows for gate chunk ci, row 32ci+30 the bias chunk; cols 128j..
    Wball = np.zeros((128, SXT * 128), dtype=np.float32)
    for j in range(SXT):
        for ci in range(4):
            csl = slice(128 * j, 128 * (j + 1))
            Wball[32 * ci + 2 * j, csl] = Wih_p[0, 128 * ci: 128 * (ci + 1)]
            Wball[32 * ci + 2 * j + 1, csl] = Wih_p[1, 128 * ci: 128 * (ci + 1)]
            Wball[32 * ci + 30, csl] = bsum_p[128 * ci: 128 * (ci + 1)]

    # xt3 per core: [128, nxt*BP]; tile til = t//SXT, slot j = t%SXT:
    # strip rows 32k+2j, 32k+2j+1 = x features (replicated over k),
    # ones row at 32k+30.
    xt3_all = np.zeros((NCORES, 128, nxt * BP), dtype=np.float16)
    xs = x.reshape(NCORES, BP, T, 2)
    xsw = np.swapaxes(xs, 1, 3)             # [NCORES, 2, T, BP]
    for t in range(T):
        til, j = divmod(t, SXT)
        csl = slice(BP * til, BP * (til + 1))
        for k in range(4):
            xt3_all[:, 32 * k + 2 * j, csl] = xsw[:, 0, t, :]
            xt3_all[:, 32 * k + 2 * j + 1, csl] = xsw[:, 1, t, :]
    for k in range(4):
        xt3_all[:, 32 * k + 30, :] = 1.0

    fc1_b = f(inputs["fc1_b"])
    fc2_W = f(inputs["fc2_W"])
    pn3_b = f(inputs["pn3_b"])
    dec1_W = f(inputs["dec1_W"])            # [131, 128]

    selS = np.zeros((3, 128), dtype=np.float32)
    selS[0, :] = 1.0
    selS[1, :] = 1.0
    selC = np.zeros((3, 128), dtype=np.float32)
    selC[2, :] = 1.0

    common = {
        "Wball": f16(Wball),
        "Whh": f16(Whh_p),
        "fc1W": f16(inputs["fc1_W"]),
        "fc1b2": f(fc1_b.reshape(2, 128).T),
        "fc2W": f16(np.concatenate([fc2_W[0:128], fc2_W[128:256]], axis=1)),
        "fc2b": f(inputs["fc2_b"][:, None]),
        "W1stk": f16(np.concatenate(
            [inputs["pn1_W"], inputs["cn1_W"]], axis=1)),   # [128, 64+64]
        "b1stk": f(np.concatenate(
            [inputs["pn1_b"], inputs["cn1_b"]])[:, None]),
        "W2stk": f16(np.concatenate(
            [inputs["pn2_W"], inputs["cn2_W"]], axis=0)),  # [128, 128]
        "pn2b": f(inputs["pn2_b"][:, None]),
        "cn2b": f(inputs["cn2_b"][:, None]),
        "pn3W": f16(inputs["pn3_W"]),        # [128, 3]
        "cn3W": f16(inputs["cn3_W"]),        # [128, 128]
        "cn3b": f(inputs["cn3_b"][:, None]),
        "sc3": np.array([[1.0], [-1.0], [-1.0]], dtype=np.float32),
        "b3": np.array([[pn3_b[0]], [-pn3_b[1]], [-pn3_b[2]]],
                       dtype=np.float32),
        "b3p": f(pn3_b[:, None]),
        "selS": f16(selS),
        "selC": f16(selC),
        "dec1aW": f16(dec1_W[0:128]),
        "dec1bW3": f16(dec1_W[128:131]),
        "dec1b": f(inputs["dec1_b"][:, None]),
        "dec2W": f16(inputs["dec2_W"]),      # [128, 64]
        "dec2b": f(inputs["dec2_b"][:, None]),
        "dec3W": f16(inputs["dec3_W"]),      # [64, 2]
        "dec3b": f(inputs["dec3_b"][:, None]),
    }

    in_maps = []
    for core in range(NCORES):
        m = dict(common)
        m["xt3"] = xt3_all[core]
        in_maps.append(m)
    return in_maps


_PROGRAM = None


def get_program():
    global _PROGRAM
    if _PROGRAM is None:
        _PROGRAM = build_program()
    return _PROGRAM


def run(inputs, **kwargs):
    nc = get_program()
    in_maps = prep_inputs(inputs)
    res = run_bass_kernel_spmd(nc, in_maps, list(range(NCORES)), **kwargs)
    outs = [res.results[i]["y"] for i in range(NCORES)]   # each [2, BP]
    y = np.concatenate([o.T for o in outs], axis=0).astype(np.float32)  # [B, 2]
    return y, res


def kernel(**inputs):
    y, _ = run(inputs)
    return y
